# revision 1
# baseline (speedup 1.0000x reference)
"""Trainium2 Bass kernel for the LogicLayer (difflogic) problem.

out[i, o] = c0[o] + ca[o]*a + cb[o]*b + cab[o]*a*b
  with a = x[i, idx_a[o]], b = x[i, idx_b[o]],
  [c0, ca, cb, cab] = softmax(weights[o]) @ GATE_COEFFS.

Strategy (8 cores, batch-sharded, 512 rows/core), batch-major:
  - x shard resident in SBUF as 4 blocks of [128, 8192].
  - gpsimd.ap_gather pulls a = x[:, idx_a-chunk], b = x[:, idx_b-chunk]
    along the free axis (indices identical for every partition).
  - combine with per-output-column coefficient tensors (pre-broadcast
    across partitions on the host, streamed from HBM per chunk):
      q = (a*cab + cb) * b;  r = a*ca + c0;  out = q + r
    6 DVE tensor_tensor ops, 2 scratch tiles (in-place updates).
"""

import numpy as np

BATCH, IN_DIM, OUT_DIM = 4096, 8192, 8192
N_CORES = 8
ROWS = BATCH // N_CORES  # 512 rows per core
P = 128
N_BB = ROWS // P         # 4 batch blocks per core
OCHUNK = 512             # output columns per chunk
N_CHUNK = OUT_DIM // OCHUNK  # 8

GATE_COEFFS = np.array([
    [0, 0, 0, 0], [0, 0, 0, 1], [0, 1, 0, -1], [0, 1, 0, 0],
    [0, 0, 1, -1], [0, 0, 1, 0], [0, 1, 1, -2], [0, 1, 1, -1],
    [1, -1, -1, 1], [1, -1, -1, 2], [1, 0, -1, 0], [1, 0, -1, 1],
    [1, -1, 0, 0], [1, -1, 0, 1], [1, 0, 0, -1], [1, 0, 0, 0],
], dtype=np.float32)  # [16, 4]

_CACHE = {}


def _build_nc(n_reps=1):
    import concourse.bacc as bacc
    import concourse.mybir as mybir
    from concourse.tile import TileContext

    f32 = mybir.dt.float32
    i16 = mybir.dt.int16
    Alu = mybir.AluOpType

    nc = bacc.Bacc("TRN2", target_bir_lowering=False, debug=False,
                   num_devices=N_CORES)
    x = nc.dram_tensor("x", [P, IN_DIM, N_BB], f32,
                       kind="ExternalInput").ap()
    idxw = nc.dram_tensor("idxw", [P, OUT_DIM // 8], i16,
                          kind="ExternalInput").ap()
    cbt = nc.dram_tensor("cbt", [P, 4, OUT_DIM], f32,
                         kind="ExternalInput").ap()
    y = nc.dram_tensor("y", [ROWS, OUT_DIM], f32, kind="ExternalOutput").ap()

    y_t = y.rearrange("(bb p) m -> bb p m", p=P)      # [4, 128, 8192]
    icols = OCHUNK // 16  # idx columns per chunk (64)

    with TileContext(nc) as tc:
        with tc.tile_pool(name="xr", bufs=1) as xpool, \
             tc.tile_pool(name="const", bufs=1) as cpool:
            xa = xpool.tile([P, IN_DIM, N_BB], f32, tag="xa")
            nc.sync.dma_start(out=xa[:], in_=x)
            idx_sb = cpool.tile([P, OUT_DIM // 8], i16, tag="idx")
            nc.sync.dma_start(out=idx_sb[:], in_=idxw)

            for rep in range(n_reps):
                with tc.tile_pool(name=f"co{rep}", bufs=2) as copool, \
                     tc.tile_pool(name=f"ab{rep}", bufs=2) as abpool, \
                     tc.tile_pool(name=f"qr{rep}", bufs=2) as qrpool:
                    for c in range(N_CHUNK):
                        cc = copool.tile([P, 4, OCHUNK], f32, tag="cc")
                        nc.sync.dma_start(
                            out=cc[:],
                            in_=cbt[:, :, c * OCHUNK:(c + 1) * OCHUNK])
                        ia = idx_sb[:, c * icols:(c + 1) * icols]
                        ib = idx_sb[:, OUT_DIM // 16 + c * icols:
                                    OUT_DIM // 16 + (c + 1) * icols]
                        c0 = cc[:, 0, :]
                        ca = cc[:, 1, :]
                        cb = cc[:, 2, :]
                        cab = cc[:, 3, :]
                        ga = abpool.tile([P, OCHUNK, N_BB], f32, tag="a")
                        nc.gpsimd.ap_gather(
                            out_ap=ga[:], in_ap=xa[:], idxs_ap=ia,
                            channels=P, num_elems=IN_DIM, d=N_BB,
                            num_idxs=OCHUNK)
                        gb = abpool.tile([P, OCHUNK, N_BB], f32, tag="b")
                        nc.gpsimd.ap_gather(
                            out_ap=gb[:], in_ap=xa[:], idxs_ap=ib,
                            channels=P, num_elems=IN_DIM, d=N_BB,
                            num_idxs=OCHUNK)
                        for bb in range(N_BB):
                            a = ga[:, :, bb]
                            b = gb[:, :, bb]
                            q = qrpool.tile([P, OCHUNK], f32, tag="q")
                            r = qrpool.tile([P, OCHUNK], f32, tag="r")
                            # q = (a*cab + cb) * b
                            nc.vector.tensor_mul(q[:], a, cab)
                            nc.vector.tensor_add(q[:], q[:], cb)
                            nc.vector.tensor_mul(q[:], q[:], b)
                            # r = a*ca + c0
                            nc.vector.tensor_mul(r[:], a, ca)
                            nc.vector.tensor_add(r[:], r[:], c0)
                            # out = q + r
                            nc.vector.tensor_add(q[:], q[:], r[:])
                            nc.sync.dma_start(
                                out=y_t[bb][:, c * OCHUNK:(c + 1) * OCHUNK],
                                in_=q[:])
    nc.compile()
    return nc


def _prep_host(x, weights, idx_a, idx_b):
    x = np.asarray(x, dtype=np.float32)
    w = np.asarray(weights, dtype=np.float32)
    e = np.exp(w - w.max(axis=1, keepdims=True))
    sm = e / e.sum(axis=1, keepdims=True)
    coeffs = (sm @ GATE_COEFFS).astype(np.float32)          # [8192, 4]
    cbt = np.ascontiguousarray(
        np.broadcast_to(coeffs.T[None, :, :], (P, 4, OUT_DIM))
    ).astype(np.float32)                                     # [128, 4, 8192]
    ia = np.asarray(idx_a).astype(np.int16)
    ib = np.asarray(idx_b).astype(np.int16)

    def wrap(seq):  # j = s*16 + p16 -> [16, len/16] -> tile to 128 partitions
        m = seq.reshape(len(seq) // 16, 16).T
        return np.tile(m, (P // 16, 1))

    idxw = np.ascontiguousarray(
        np.concatenate([wrap(ia), wrap(ib)], axis=1))        # [128, 1024]
    xi = []
    for c in range(N_CORES):
        sh = x[c * ROWS:(c + 1) * ROWS]          # [512, 8192]
        xi.append(np.ascontiguousarray(
            sh.reshape(N_BB, P, IN_DIM).transpose(1, 2, 0)))  # [128,8192,4]
    return xi, idxw, cbt


def _in_maps(x, weights, idx_a, idx_b):
    xi, idxw, cbt = _prep_host(x, weights, idx_a, idx_b)
    return [{"x": xi[c], "idxw": idxw, "cbt": cbt}
            for c in range(N_CORES)]


def kernel(x, weights, idx_a, idx_b):
    from concourse.bass_utils import run_bass_kernel_spmd

    in_maps = _in_maps(x, weights, idx_a, idx_b)
    if "nc" not in _CACHE:
        _CACHE["nc"] = _build_nc()
    nc = _CACHE["nc"]
    res = run_bass_kernel_spmd(nc, in_maps, list(range(N_CORES)))
    out = np.concatenate([res.results[c]["y"] for c in range(N_CORES)], axis=0)
    return out.astype(np.float32)



# revision 7
# speedup vs baseline: 4.5653x; 4.5653x over previous
"""Trainium2 Bass kernel for the LogicLayer (difflogic) problem.

out[i, o] = c0[o] + ca[o]*a + cb[o]*b + cab[o]*a*b
  with a = x[i, idx_a[o]], b = x[i, idx_b[o]],
  [c0, ca, cb, cab] = softmax(weights[o]) @ GATE_COEFFS.

Strategy (8 cores, batch-sharded, 512 rows/core), output-major:
  - x shard transposed on host to xt[8192 feat, 512 rows] fp16 in HBM.
  - per rep: SWDGE dma_gather pulls rows xt[idx_a], xt[idx_b] into
    SBUF tiles [128 outs, 512 rows] (runs on the 16 SDMA engines at
    ~22 B/ns each; 1 KiB per descriptor).
  - per 128-output block, coefficients are per-partition scalars:
      t = a*cab + cb     (DVE tensor_scalar, 4x mode)
      r = a*ca  + c0     (ACT Identity activation, parallel engine)
      t = t*b; o = t + r (DVE tensor_tensor, 2x mode)
  - y written [8192 outs, 512 rows] fp16; host transposes/casts back.
"""

import numpy as np

BATCH, IN_DIM, OUT_DIM = 4096, 8192, 8192
N_CORES = 8
ROWS = BATCH // N_CORES   # 512 rows per core
P = 128
NBLK = OUT_DIM // P       # 64 output blocks per core
NQ = 8                    # gather chunks per operand (num_idxs<=1024: HW limit)
OQ = OUT_DIM // NQ        # 1024 outputs per gather
BPQ = NBLK // NQ          # 16 blocks per quarter
ICOLS = OQ // 16          # idx columns per gather chunk (128)

GATE_COEFFS = np.array([
    [0, 0, 0, 0], [0, 0, 0, 1], [0, 1, 0, -1], [0, 1, 0, 0],
    [0, 0, 1, -1], [0, 0, 1, 0], [0, 1, 1, -2], [0, 1, 1, -1],
    [1, -1, -1, 1], [1, -1, -1, 2], [1, 0, -1, 0], [1, 0, -1, 1],
    [1, -1, 0, 0], [1, -1, 0, 1], [1, 0, 0, -1], [1, 0, 0, 0],
], dtype=np.float32)  # [16, 4]

_CACHE = {}


def _build_nc(n_reps=1):
    import concourse.bacc as bacc
    import concourse.mybir as mybir
    from concourse.tile import TileContext

    f32 = mybir.dt.float32
    f16 = mybir.dt.float16
    i16 = mybir.dt.int16
    Alu = mybir.AluOpType
    Act = mybir.ActivationFunctionType

    nc = bacc.Bacc("TRN2", target_bir_lowering=False, debug=False,
                   num_devices=N_CORES)
    xt = nc.dram_tensor("xt", [IN_DIM, ROWS], f16, kind="ExternalInput").ap()
    idxw = nc.dram_tensor("idxw", [P, 2 * NQ * ICOLS], i16,
                          kind="ExternalInput").ap()
    coef = nc.dram_tensor("coef", [P, 4, NBLK], f32,
                          kind="ExternalInput").ap()
    y = nc.dram_tensor("y", [OUT_DIM, ROWS], f16, kind="ExternalOutput").ap()

    with TileContext(nc) as tc:
        with tc.tile_pool(name="const", bufs=1) as cpool, \
             tc.tile_pool(name="gab", bufs=4) as gpool, \
             tc.tile_pool(name="tr", bufs=4) as tpool, \
             tc.tile_pool(name="out", bufs=4) as opool:
            idx_sb = cpool.tile([P, 2 * NQ * ICOLS], i16, tag="idx")
            nc.sync.dma_start(out=idx_sb[:], in_=idxw)
            cf = cpool.tile([P, 4, NBLK], f32, tag="coef")
            nc.sync.dma_start(out=cf[:], in_=coef)

            for rep in range(n_reps):
                for q in range(NQ):
                    ga = gpool.tile([P, BPQ, ROWS], f16, tag="ga")
                    nc.gpsimd.dma_gather(
                        ga[:], xt, idx_sb[:, q * ICOLS:(q + 1) * ICOLS],
                        OQ, OQ, ROWS)
                    gb = gpool.tile([P, BPQ, ROWS], f16, tag="gb")
                    nc.gpsimd.dma_gather(
                        gb[:], xt,
                        idx_sb[:, (NQ + q) * ICOLS:(NQ + q + 1) * ICOLS],
                        OQ, OQ, ROWS)
                    for j in range(BPQ):
                        m = q * BPQ + j
                        a = ga[:, j, :]
                        b = gb[:, j, :]
                        t = tpool.tile([P, ROWS], f16, tag="t")
                        nc.vector.tensor_scalar(
                            t[:], a, cf[:, 3, m:m + 1], cf[:, 2, m:m + 1],
                            Alu.mult, Alu.add)
                        r = tpool.tile([P, ROWS], f16, tag="r")
                        nc.scalar.activation(
                            r[:], a, Act.Identity,
                            bias=cf[:, 0, m:m + 1], scale=cf[:, 1, m:m + 1])
                        nc.vector.tensor_mul(t[:], t[:], b)
                        o = opool.tile([P, ROWS], f16, tag="o")
                        nc.vector.tensor_add(o[:], t[:], r[:])
                        nc.sync.dma_start(
                            out=y[m * P:(m + 1) * P, :], in_=o[:])
    nc.compile()
    return nc


def _wrap_idx(seq):
    # dma_gather index layout: unwrapped[i] = idxs[i % 16, i // 16],
    # tiled to 128 partitions (replicated across the 8 Q7 cores).
    m = seq.reshape(len(seq) // 16, 16).T
    return np.tile(m, (P // 16, 1))


def _prep_host(x, weights, idx_a, idx_b):
    x = np.asarray(x, dtype=np.float32)
    w = np.asarray(weights, dtype=np.float32)
    e = np.exp(w - w.max(axis=1, keepdims=True))
    sm = e / e.sum(axis=1, keepdims=True)
    coeffs = (sm @ GATE_COEFFS).astype(np.float32)          # [8192, 4]
    # coef[p, k, m] = coeffs[o, k] with o = (m // BPQ)*OQ + (m % BPQ)*128 + p
    cf = coeffs.reshape(NQ, BPQ, P, 4).transpose(2, 3, 0, 1)  # [128,4,NQ,BPQ]
    cf = np.ascontiguousarray(cf.reshape(P, 4, NBLK))

    ia = np.asarray(idx_a).astype(np.int16)
    ib = np.asarray(idx_b).astype(np.int16)
    cols = [_wrap_idx(ia[q * OQ:(q + 1) * OQ]) for q in range(NQ)]
    cols += [_wrap_idx(ib[q * OQ:(q + 1) * OQ]) for q in range(NQ)]
    idxw = np.ascontiguousarray(np.concatenate(cols, axis=1))  # [128, 1024]

    xts = []
    for c in range(N_CORES):
        sh = x[c * ROWS:(c + 1) * ROWS]                     # [512, 8192]
        xts.append(np.ascontiguousarray(sh.T.astype(np.float16)))
    return xts, idxw, cf


def _in_maps(x, weights, idx_a, idx_b):
    xts, idxw, cf = _prep_host(x, weights, idx_a, idx_b)
    return [{"xt": xts[c], "idxw": idxw, "coef": cf}
            for c in range(N_CORES)]


def kernel(x, weights, idx_a, idx_b):
    from concourse.bass_utils import run_bass_kernel_spmd

    in_maps = _in_maps(x, weights, idx_a, idx_b)
    if "nc" not in _CACHE:
        _CACHE["nc"] = _build_nc()
    nc = _CACHE["nc"]
    res = run_bass_kernel_spmd(nc, in_maps, list(range(N_CORES)))
    out = np.concatenate(
        [res.results[c]["y"].T.astype(np.float32) for c in range(N_CORES)],
        axis=0)
    return out


# revision 8
# speedup vs baseline: 8.6016x; 1.8841x over previous
"""Trainium2 Bass kernel for the LogicLayer (difflogic) problem.

out[i, o] = c0[o] + ca[o]*a + cb[o]*b + cab[o]*a*b
  with a = x[i, idx_a[o]], b = x[i, idx_b[o]],
  [c0, ca, cb, cab] = softmax(weights[o]) @ GATE_COEFFS.

Strategy: OUTPUT-sharded across 8 cores (1024 outputs/core, all 4096
batch rows). x transposed on host to xt[8192 feat, 4096 rows] fp16 in
HBM (replicated). Rationale: SWDGE descriptor generation costs ~9ns per
gather index on the Q7s, so gather few (2048/rep) large (8 KiB) rows
rather than many small ones; the 16 SDMA engines then stream at full
bandwidth.

Per rep and core:
  - 8x dma_gather (256 idx each) pull xt[idx_a[o]] / xt[idx_b[o]] rows
    into SBUF tiles [128 outs, 4096 rows] fp16.
  - per 128-output block (coefficients are per-partition scalars):
      t = a*cab + cb     (DVE tensor_scalar dual-op, 4x mode, 1.3us)
      r = a*ca  + c0     (ACT Identity activation, parallel engine)
      t = t*b; o = t + r (DVE tensor_tensor, 2x mode, 2.3us each)
  - y written [1024 outs, 4096 rows] fp16; host transposes/casts back.
"""

import numpy as np

BATCH, IN_DIM, OUT_DIM = 4096, 8192, 8192
N_CORES = 8
OPC = OUT_DIM // N_CORES  # 1024 outputs per core
RA = BATCH                # all 4096 rows per core
P = 128
NBLK = OPC // P           # 8 output blocks per core
NI = 256                  # indices per dma_gather
NCH = OPC // NI           # 4 gather chunks per operand
BPC = NI // P             # 2 blocks per chunk
ICOLS = NI // 16          # idx columns per chunk (16)

GATE_COEFFS = np.array([
    [0, 0, 0, 0], [0, 0, 0, 1], [0, 1, 0, -1], [0, 1, 0, 0],
    [0, 0, 1, -1], [0, 0, 1, 0], [0, 1, 1, -2], [0, 1, 1, -1],
    [1, -1, -1, 1], [1, -1, -1, 2], [1, 0, -1, 0], [1, 0, -1, 1],
    [1, -1, 0, 0], [1, -1, 0, 1], [1, 0, 0, -1], [1, 0, 0, 0],
], dtype=np.float32)  # [16, 4]

_CACHE = {}


def _build_nc(n_reps=1):
    import concourse.bacc as bacc
    import concourse.mybir as mybir
    from concourse.tile import TileContext

    f32 = mybir.dt.float32
    f16 = mybir.dt.float16
    i16 = mybir.dt.int16
    Alu = mybir.AluOpType
    Act = mybir.ActivationFunctionType

    nc = bacc.Bacc("TRN2", target_bir_lowering=False, debug=False,
                   num_devices=N_CORES)
    xt = nc.dram_tensor("xt", [IN_DIM, RA], f16, kind="ExternalInput").ap()
    idxw = nc.dram_tensor("idxw", [P, 2 * NCH * ICOLS], i16,
                          kind="ExternalInput").ap()
    coef = nc.dram_tensor("coef", [P, 4, NBLK], f32,
                          kind="ExternalInput").ap()
    y = nc.dram_tensor("y", [OPC, RA], f16, kind="ExternalOutput").ap()

    with TileContext(nc) as tc:
        with tc.tile_pool(name="const", bufs=1) as cpool, \
             tc.tile_pool(name="gab", bufs=3) as gpool, \
             tc.tile_pool(name="tr", bufs=2) as tpool, \
             tc.tile_pool(name="out", bufs=2) as opool:
            idx_sb = cpool.tile([P, 2 * NCH * ICOLS], i16, tag="idx")
            nc.sync.dma_start(out=idx_sb[:], in_=idxw)
            cf = cpool.tile([P, 4, NBLK], f32, tag="coef")
            nc.sync.dma_start(out=cf[:], in_=coef)

            for rep in range(n_reps):
                for q in range(NCH):
                    ga = gpool.tile([P, BPC, RA], f16, tag="ga")
                    nc.gpsimd.dma_gather(
                        ga[:], xt, idx_sb[:, q * ICOLS:(q + 1) * ICOLS],
                        NI, NI, RA)
                    gb = gpool.tile([P, BPC, RA], f16, tag="gb")
                    nc.gpsimd.dma_gather(
                        gb[:], xt,
                        idx_sb[:, (NCH + q) * ICOLS:(NCH + q + 1) * ICOLS],
                        NI, NI, RA)
                    for j in range(BPC):
                        m = q * BPC + j
                        a = ga[:, j, :]
                        b = gb[:, j, :]
                        t = tpool.tile([P, RA], f16, tag="t")
                        nc.vector.tensor_scalar(
                            t[:], a, cf[:, 3, m:m + 1], cf[:, 2, m:m + 1],
                            Alu.mult, Alu.add)
                        r = tpool.tile([P, RA], f16, tag="r")
                        nc.scalar.activation(
                            r[:], a, Act.Identity,
                            bias=cf[:, 0, m:m + 1], scale=cf[:, 1, m:m + 1])
                        nc.vector.tensor_mul(t[:], t[:], b)
                        o = opool.tile([P, RA], f16, tag="o")
                        nc.vector.tensor_add(o[:], t[:], r[:])
                        nc.sync.dma_start(
                            out=y[m * P:(m + 1) * P, :], in_=o[:])
    nc.compile()
    return nc


def _wrap_idx(seq):
    # dma_gather index layout: unwrapped[i] = idxs[i % 16, i // 16],
    # tiled to 128 partitions (replicated across the 8 Q7 cores).
    m = seq.reshape(len(seq) // 16, 16).T
    return np.tile(m, (P // 16, 1))


def _prep_host(x, weights, idx_a, idx_b):
    x = np.asarray(x, dtype=np.float32)
    w = np.asarray(weights, dtype=np.float32)
    e = np.exp(w - w.max(axis=1, keepdims=True))
    sm = e / e.sum(axis=1, keepdims=True)
    coeffs = (sm @ GATE_COEFFS).astype(np.float32)          # [8192, 4]

    xt = np.ascontiguousarray(x.T.astype(np.float16))       # [8192, 4096]
    ia = np.asarray(idx_a).astype(np.int16)
    ib = np.asarray(idx_b).astype(np.int16)

    idxws, cfs = [], []
    for c in range(N_CORES):
        lo, hi = c * OPC, (c + 1) * OPC
        cols = [_wrap_idx(ia[lo + q * NI:lo + (q + 1) * NI])
                for q in range(NCH)]
        cols += [_wrap_idx(ib[lo + q * NI:lo + (q + 1) * NI])
                 for q in range(NCH)]
        idxws.append(np.ascontiguousarray(np.concatenate(cols, axis=1)))
        # coef[p, k, m] = coeffs[lo + m*128 + p, k]
        cf = coeffs[lo:hi].reshape(NBLK, P, 4).transpose(1, 2, 0)
        cfs.append(np.ascontiguousarray(cf))
    return xt, idxws, cfs


def _in_maps(x, weights, idx_a, idx_b):
    xt, idxws, cfs = _prep_host(x, weights, idx_a, idx_b)
    return [{"xt": xt, "idxw": idxws[c], "coef": cfs[c]}
            for c in range(N_CORES)]


def kernel(x, weights, idx_a, idx_b):
    from concourse.bass_utils import run_bass_kernel_spmd

    in_maps = _in_maps(x, weights, idx_a, idx_b)
    if "nc" not in _CACHE:
        _CACHE["nc"] = _build_nc()
    nc = _CACHE["nc"]
    res = run_bass_kernel_spmd(nc, in_maps, list(range(N_CORES)))
    out = np.concatenate(
        [res.results[c]["y"].T.astype(np.float32) for c in range(N_CORES)],
        axis=1)
    return out
